# revision 17
# baseline (speedup 1.0000x reference)
"""StyleGAN2 conv_downsample_2d (FIR [1,3,3,1] + strided 1x1 conv) on 8 TRN2 cores.

Math (NCHW, per sample n):
    out[co, i, j] = sum_ci w[ci,co] * sum_{dy,dx} K2D[dy,dx] * x[ci, 2i+dy-1, 2j+dx-1]
with K2D = outer(k,k)/64, k = [1,3,3,1]  (symmetric, so the spatial flip is a no-op).

The kernel is HBM-bandwidth bound (measured: baseline fp32 moved 101.8 MB/core at
exactly 358 GB/s line rate), so all device traffic is fp16: the host casts + packs
the input shard, and the device writes an fp16 output that the host upcasts.
rel-err budget: fp16 quantization ~1e-3 of absmax, well under the 2e-2 gate.

Decomposition per core (data-parallel over (sample, H-half) -> 8 shards):
  1. Host packs each row in column-polyphase order [0 | odd cols | even cols | 0]
     (width 514). The two zero columns pre-pad the horizontal FIR, and after the
     (per-column) vertical FIR the row IS the polyphase pair (ve | vo), so every
     DVE op below is a contiguous full-width op (fp16 2x_1P mode).
  2. Vertical 4-tap FIR at row-stride 2 on VectorE: s = x1+x2, t = x0+x3,
     v = 3*s + t (one scalar_tensor_tensor over the full 514-wide row).
  3. Horizontal FIR + channel mix fused on TensorE: 4 PSUM-accumulating fp16
     matmuls per output tile; tap dx reads v at col offset {0, 257, 1, 258}
     (= v_pad[2j+dx] in polyphase layout); lhsT = w * k[dx]/64 (host-precomputed).
  4. PSUM (fp32) -> SBUF stage with fp16 cast on ScalarE, DMA out fp16.

Each shard is host-padded to a uniform [128, 258, 514] window so all 8 cores run
the identical SPMD program (no partition-id branching).
"""

import numpy as np

import concourse.bass as bass
import concourse.mybir as mybir
from concourse import bacc
from concourse.tile import TileContext
from concourse.bass_utils import run_bass_kernel_spmd

N_CORES = 8
C_IN = 128
C_OUT = 256
H = 512
W = 512
HO = 256  # full output rows; 128 per core
WO = 256
SHARD_ROWS = 258  # 2*128 rows of taps + 2 boundary rows (host zero-padded)
TILE_ROWS = 16
N_TILES = 17  # 16 full 16-row tiles + one 2-row tail tile
N_CHUNKS = 16  # v-chunks of 8 output rows -> 128 output rows per core
VW = 514  # packed row: [zero | 256 odd cols | 256 even cols | zero]

F16 = mybir.dt.float16
F32 = mybir.dt.float32

_CACHED_NC = None


def _build_program():
    nc = bacc.Bacc("TRN2", target_bir_lowering=False)

    x = nc.dram_tensor("x", [C_IN, SHARD_ROWS, VW], F16, kind="ExternalInput")
    wp = nc.dram_tensor("wp", [C_IN, 4, 2, 128], F16, kind="ExternalInput")
    out = nc.dram_tensor("out", [C_OUT, HO // 2, WO], F16, kind="ExternalOutput")

    with TileContext(nc) as tc:
        with (
            tc.tile_pool(name="inp", bufs=6) as inp_pool,
            tc.tile_pool(name="vpool", bufs=2) as v_pool,
            tc.tile_pool(name="stpool", bufs=1) as st_pool,
            tc.tile_pool(name="stage", bufs=3) as stage_pool,
            tc.tile_pool(name="wpool", bufs=1) as w_pool,
            tc.tile_pool(name="psum", bufs=2, space="PSUM") as psum_pool,
        ):
            wsb = w_pool.tile([C_IN, 4, 2, 128], F16, tag="w")

            # s/t pair-sum scratch: fully rewritten every chunk (no carried
            # state -> no cross-chunk scheduling dependencies).
            s = st_pool.tile([C_IN, 8, VW], F16, tag="s")
            t = st_pool.tile([C_IN, 8, VW], F16, tag="t")
            s3 = st_pool.tile([C_IN, 8, VW], F16, tag="s3")

            tiles: dict[int, object] = {}

            def in_tile(k):
                if k not in tiles:
                    t = inp_pool.tile([C_IN, TILE_ROWS, VW], F16, tag="in")
                    rows = 2 if k == N_TILES - 1 else TILE_ROWS
                    if k == 0:
                        # Two-piece first tile: chunk 0's first 4-row block
                        # only needs rows 0..9, so compute starts ~3us sooner.
                        nc.sync.dma_start(out=t[:, 0:10, :], in_=x[:, 0:10, :])
                        nc.sync.dma_start(out=t[:, 10:16, :], in_=x[:, 10:16, :])
                    else:
                        nc.sync.dma_start(
                            out=t[:, 0:rows, :],
                            in_=x[:, TILE_ROWS * k : TILE_ROWS * k + rows, :],
                        )
                    tiles[k] = t
                return tiles[k]

            # Queue the first two input tiles ahead of the weight load so the
            # HBM read stream starts at the earliest possible point.
            in_tile(0)
            in_tile(1)
            nc.sync.dma_start(out=wsb[:], in_=wp[:])

            # rhs col offset into the polyphase v row per horizontal tap dx:
            # v_pad[2j+dx] = v[OFF[dx] + j]  (ve at 0..256, vo at 257..513).
            OFF = [0, 257, 1, 258]

            # out viewed as [co_local=128, half, row, col] so one DMA can write
            # both co-halves of a chunk from a single stage tile.
            out_hv = out.rearrange("(h co) i j -> co h i j", h=2)

            def emit_block(vrow0, nrows, ta, tb, roff, stage, srow0, stage_rows):
                """One v-block: v rows [vrow0, vrow0+nrows). Tap m (0..nrows)
                reads tile `ta` local rows roff+2m .. roff+2m+3, spilling into
                the first two rows of tile `tb` when past row 15. Results land
                in stage rows [srow0, srow0+nrows); the caller DMAs the stage."""
                v = v_pool.tile([C_IN, nrows, VW], F16, tag="v")

                # s[m] = x[2m+1] + x[2m+2]   (middle taps, weight 3)
                # t[m] = x[2m] + x[2m+3]     (outer taps, weight 1)
                ms = min(nrows, (13 - roff) // 2 + 1)  # rows with 2m+2+roff <= 15
                mt = min(nrows, (12 - roff) // 2 + 1)  # rows with 2m+3+roff <= 15
                nc.vector.tensor_add(
                    out=s[:, 0:ms, :],
                    in0=ta[:, roff + 1 : roff + 2 * ms : 2, :],
                    in1=ta[:, roff + 2 : roff + 2 * ms + 1 : 2, :],
                )
                if ms < nrows:  # single boundary row: x[15] + next[0]
                    # Boundary rows run on the (otherwise idle) GpSimd engine,
                    # keeping the DVE — the compute pacer — on the bulk rows.
                    nc.gpsimd.tensor_add(
                        out=s[:, ms : ms + 1, :], in0=ta[:, 15:16, :], in1=tb[:, 0:1, :]
                    )
                nc.vector.tensor_add(
                    out=t[:, 0:mt, :],
                    in0=ta[:, roff : roff + 2 * mt - 1 : 2, :],
                    in1=ta[:, roff + 3 : roff + 2 * mt + 2 : 2, :],
                )
                if mt < nrows:  # single boundary row: x[14] + next[1]
                    nc.gpsimd.tensor_add(
                        out=t[:, mt : mt + 1, :], in0=ta[:, 14:15, :], in1=tb[:, 1:2, :]
                    )

                # v = 3*s + t over the full packed row (the zero pad columns
                # stay zero). scalar_tensor_tensor only has a 1x DVE uop, so
                # split it: tensor_scalar (4x uop for 16-bit) + tensor_add (2x)
                # is 3200 cy vs 4170 for the fused 1x op.
                nc.vector.tensor_scalar_mul(s3[:, 0:nrows, :], s[:, 0:nrows, :], 3.0)
                nc.vector.tensor_add(
                    out=v[:, 0:nrows, :], in0=s3[:, 0:nrows, :], in1=t[:, 0:nrows, :]
                )

                # Horizontal FIR + 1x1 conv: out[co, m, j] = sum_dx lhsT_dx.T @ v_pad[., 2j+dx]
                for half in range(2):
                    # One multi-bank PSUM tile per half: each row-pair's 4-tap
                    # accumulation group lands in its own (bank-aligned) 2KB
                    # slice, and the whole tile drains with a single ACT copy
                    # (per-op bubble would dominate with per-bank copies).
                    p = psum_pool.tile([128, nrows, WO], F32, tag="ps")
                    for rp in range(nrows // 2):
                        for dx in range(4):
                            nc.tensor.matmul(
                                p[:, 2 * rp : 2 * rp + 2, :],
                                wsb[:, dx, half, :],
                                v[:, 2 * rp : 2 * rp + 2, OFF[dx] : OFF[dx] + 256],
                                start=(dx == 0),
                                stop=(dx == 3),
                            )
                    nc.scalar.copy(
                        out=stage[:, half, srow0 : srow0 + nrows], in_=p[:]
                    )

            def stage_tile(rows):
                return stage_pool.tile(
                    [128, 2, rows, WO], F16, tag="stage", name="stage"
                )

            def stage_dma(stage, vrow0, rows):
                # Issued on the ACT HWDGE ring so output transfers don't queue
                # FIFO behind not-yet-needed input tiles on the SP ring.
                nc.scalar.dma_start(
                    out=out_hv[:, :, vrow0 : vrow0 + rows, :], in_=stage[:]
                )

            def chunk(c, sub):
                """Emit chunk c as len(sub) blocks of 8/sub rows each."""
                n = 8 // sub
                for k in range(sub):
                    stage = stage_tile(n)
                    tb = in_tile(c + 1) if k == sub - 1 else None
                    emit_block(
                        8 * c + n * k, n, in_tile(c), tb, 2 * n * k, stage, 0, n
                    )
                    stage_dma(stage, 8 * c + n * k, n)

            # Chunk 0 in 4-row blocks (starts after only 10 input rows);
            # steady-state chunks whole (1MB output DMAs measured best);
            # the final two chunks in 4/2-row blocks to shorten the
            # end-of-kernel drain after the input stream finishes.
            chunk(0, 2)
            for c in range(1, N_CHUNKS - 2):
                chunk(c, 1)
            chunk(N_CHUNKS - 2, 2)
            chunk(N_CHUNKS - 1, 4)
    nc.finalize()
    return nc


def _get_nc():
    global _CACHED_NC
    if _CACHED_NC is None:
        _CACHED_NC = _build_program()
    return _CACHED_NC


def _prep_inputs(images, w):
    images = np.asarray(images, dtype=np.float32)
    w = np.asarray(w, dtype=np.float32)
    assert images.shape == (4, C_IN, H, W), images.shape
    assert w.shape == (1, 1, C_IN, C_OUT), w.shape

    k = np.array([1.0, 3.0, 3.0, 1.0], dtype=np.float32)
    # wq[ci, dx, half, co] = w[ci, 128*half+co] * k[dx] / 64
    wq = np.ascontiguousarray(
        w[0, 0].reshape(C_IN, 1, 2, 128) * (k / 64.0).reshape(1, 4, 1, 1)
    ).astype(np.float16)

    in_maps = []
    for n in range(4):
        for half in range(2):
            # half 0: padded global rows -1..256 ; half 1: padded global rows 255..512
            packed = np.zeros((C_IN, SHARD_ROWS, VW), dtype=np.float16)
            if half == 0:
                src, r0, r1 = images[n][:, 0:257, :], 1, 258
            else:
                src, r0, r1 = images[n][:, 255:512, :], 0, 257
            packed[:, r0:r1, 1:257] = src[:, :, 1::2]  # odd cols -> ve[1:]
            packed[:, r0:r1, 257:513] = src[:, :, 0::2]  # even cols -> vo[:-1]
            in_maps.append({"x": packed, "wp": wq})
    return in_maps


def _assemble(results):
    out = np.empty((4, C_OUT, HO, WO), dtype=np.float32)
    for n in range(4):
        for half in range(2):
            out[n, :, 128 * half : 128 * (half + 1), :] = results[2 * n + half]["out"]
    return out


def run(images, w, **spmd_kwargs):
    """Full pipeline; returns (output, BassKernelResults)."""
    nc = _get_nc()
    in_maps = _prep_inputs(images, w)
    res = run_bass_kernel_spmd(nc, in_maps, core_ids=list(range(N_CORES)), **spmd_kwargs)
    return _assemble(res.results), res


def kernel(images, w):
    out, _ = run(images, w)
    return out
